# revision 1
# baseline (speedup 1.0000x reference)
"""GAT (2-layer, 4-head) regressor on 8 Trainium2 NeuronCores.

Strategy (dst-owner graph partition, per spec sharding_hint):
  * Host packs nodes into 8 cores x 98 blocks of 128 dst slots, balancing
    per-(block, src-chunk) edge counts into fixed-capacity gather slabs.
  * Each core redundantly computes the full layer-1 feature table
    (h = x @ W1, plus fused per-head attention dots) into local DRAM as
    512-byte rows [h bf16 x128 | alpha_src fp32 x4 | alpha_dst fp32 x4 | pad].
  * Edges (grouped by dst block, src chunk) gather rows with dma_gather
    (int16 indices into 25088-row chunk views).
  * Per 128-edge tile: dst-indicator matrix M2 via tensor_scalar is_equal,
    M1 = M2^T on TensorE, alpha_dst expand matmul, softmax (no max-sub --
    logits bounded ~8), weighted aggregation matmul with fused denominator
    column.  Per-dst normalize + bias + exact ELU epilogue.
  * Layer-2 table rows are computed from own y1 tiles and AllGathered;
    layer 2 repeats the edge phase; FC head emits [12544, 2] per core.
"""

import os
import sys
import time

for _p in ("/opt/trn_rl_repo", "/root/.axon_site/_ro/trn_rl_repo"):
    if os.path.isdir(_p) and _p not in sys.path:
        sys.path.append(_p)

import numpy as np
import ml_dtypes

from concourse import bacc, bass, mybir, tile, library_config
from concourse.bass_utils import run_bass_kernel_spmd

F32 = mybir.dt.float32
BF16 = mybir.dt.bfloat16
I16 = mybir.dt.int16
U16 = mybir.dt.uint16
OP = mybir.AluOpType
AF = mybir.ActivationFunctionType

P = 128
HEADS, HID = 4, 32
FEAT = HEADS * HID          # 128
FA = FEAT + 2 * HEADS       # 136
ROWW = 256                  # uint16 units per table row (512 B)
NCORES = 8


class Cfg:
    def __init__(self, n_nodes, nblk, caps_base):
        self.N = n_nodes
        self.NBLK = nblk                      # blocks per core
        self.NSLOT = nblk * P                 # dst slots per core
        self.NTOT = NCORES * self.NSLOT       # global slots
        self.CHUNK = self.NTOT // 4           # src chunk (2 cores) for int16 idx
        assert self.CHUNK == 2 * self.NSLOT and self.CHUNK < 32768
        self.caps_base = caps_base            # e.g. [5,5,5,4]; rotated by block%4
        self.TBLK = sum(caps_base)            # tiles per block
        self.NTILE = nblk * self.TBLK         # tiles per layer per core

    def caps(self, b):
        r = b % 4
        cb = self.caps_base
        # rotate so that the small cap cycles through chunks
        return [cb[(c - r) % 4] for c in range(4)]


REAL = Cfg(100000, 98, [5, 5, 4, 4])


# --------------------------------------------------------------------------
# host-side packing
# --------------------------------------------------------------------------

def _assign_blocks(cfg, deg4, nodes, seed):
    """Pack `nodes` (one core's) into NBLK blocks of <=128 dsts s.t. per-chunk
    edge loads fit cfg.caps(b)*128.  Returns block id per node or None."""
    rng = np.random.default_rng(seed)
    nblk = cfg.NBLK
    caps = np.array([cfg.caps(b) for b in range(nblk)], np.int64) * P
    loads = np.zeros((nblk, 4), np.int64)
    counts = np.zeros(nblk, np.int64)
    order = np.argsort(-deg4[nodes].sum(1), kind="stable")
    blk_of = np.empty(len(nodes), np.int64)
    for i in order:
        d = deg4[nodes[i]]
        new = loads + d
        feas = (counts < P) & (new <= caps).all(1)
        if not feas.any():
            return None
        frac = (new / caps).max(1)
        slack = (P - counts) / P
        frac = np.where(feas, frac - 1e-4 * slack, np.inf)
        b = int(np.argmin(frac))
        blk_of[i] = b
        loads[b] += d
        counts[b] += 1
    return blk_of


def pack(cfg, x, edge_index, seed=0):
    t0 = time.time()
    N = cfg.N
    ei = np.asarray(edge_index)
    src = ei[0].astype(np.int64)
    dst = ei[1].astype(np.int64)

    rng = np.random.default_rng(seed)
    perm = rng.permutation(N)
    core_of = np.empty(N, np.int64)
    per_core = N // NCORES
    for k in range(NCORES):
        core_of[perm[k * per_core:(k + 1) * per_core]] = k
    chunk_of_node = core_of // 2

    # per-node per-chunk in-degree (self loops handled densely on-chip)
    key = dst * 4 + chunk_of_node[src]
    deg4 = np.bincount(key, minlength=4 * N).reshape(N, 4)

    slot_of = np.full(N, -1, np.int64)
    for k in range(NCORES):
        nodes = perm[k * per_core:(k + 1) * per_core]
        blk = _assign_blocks(cfg, deg4, nodes, seed + k)
        assert blk is not None, "block packing failed; bump caps"
        order = np.lexsort((nodes, blk))
        local = np.empty(len(nodes), np.int64)
        pos = 0
        prev = -1
        for j in order:
            if blk[j] != prev:
                pos = 0
                prev = blk[j]
            local[j] = pos
            pos += 1
            assert pos <= P
        slot_of[nodes] = k * cfg.NSLOT + blk * P + local

    node_of_slot = np.full(cfg.NTOT, -1, np.int64)
    node_of_slot[slot_of] = np.arange(N)

    s_slot = slot_of[src]
    d_slot = slot_of[dst]
    e_core = d_slot // cfg.NSLOT
    e_blk = (d_slot % cfg.NSLOT) // P
    e_chunk = s_slot // cfg.CHUNK
    e_dl = d_slot % P

    # per-core arrays
    idx_all, dst_all = [], []
    okey = ((e_core * cfg.NBLK + e_blk) * 4 + e_chunk) * 200000 + e_dl
    eorder = np.argsort(okey, kind="stable")
    s_sorted = s_slot[eorder]
    grp = (e_core * cfg.NBLK + e_blk)[eorder] * 4 + e_chunk[eorder]
    dl_sorted = e_dl[eorder]
    bounds = np.searchsorted(grp, np.arange(NCORES * cfg.NBLK * 4 + 1))

    for k in range(NCORES):
        idx_parts, dst_parts = [], []
        for b in range(cfg.NBLK):
            caps = cfg.caps(b)
            for c in range(4):
                g = (k * cfg.NBLK + b) * 4 + c
                lo, hi = bounds[g], bounds[g + 1]
                n = hi - lo
                cap = caps[c] * P
                assert n <= cap, (k, b, c, n, cap)
                iloc = np.zeros(cap, np.int16)
                dloc = np.full(cap, 255.0, np.float32)
                iloc[:n] = (s_sorted[lo:hi] - c * cfg.CHUNK).astype(np.int16)
                dloc[:n] = dl_sorted[lo:hi].astype(np.float32)
                idx_parts.append(np.tile(iloc.reshape(-1, 16).T, (8, 1)))
                dst_parts.append(dloc.reshape(caps[c], P).T)
        idx_all.append(np.concatenate(idx_parts, axis=1).astype(np.int16))
        dst_all.append(np.concatenate(dst_parts, axis=1).astype(np.float32))

    # node-feature table input, transposed, slot order, bf16
    xT = np.zeros((P, cfg.NTOT), dtype=ml_dtypes.bfloat16)
    xs = np.asarray(x)[node_of_slot.clip(0)].astype(ml_dtypes.bfloat16)
    xs[node_of_slot < 0] = 0
    xT[:, :] = xs.T
    print(f"[pack] {time.time()-t0:.1f}s", flush=True)
    return {
        "idx": idx_all, "dstcol": dst_all, "xT": xT,
        "node_of_slot": node_of_slot, "slot_of": slot_of,
    }


def make_weights(cfg, W1, a_src1, a_dst1, b1, W2, a_src2, a_dst2, b2, Wfc, bfc):
    def amat(a_s, a_d):
        A = np.zeros((FEAT, 8), np.float32)
        for h in range(HEADS):
            A[h * HID:(h + 1) * HID, h] = np.asarray(a_s)[h]
            A[h * HID:(h + 1) * HID, 4 + h] = np.asarray(a_d)[h]
        return A

    W1 = np.asarray(W1, np.float32)
    W2 = np.asarray(W2, np.float32)
    W1p = np.concatenate([W1, W1 @ amat(a_src1, a_dst1)], 1).astype(ml_dtypes.bfloat16)
    W2p = np.concatenate([W2, W2 @ amat(a_src2, a_dst2)], 1).astype(ml_dtypes.bfloat16)
    consts = {
        "W1p": W1p, "W2p": W2p,
        "Wfc": np.asarray(Wfc, np.float32),
        "b1b": np.broadcast_to(np.asarray(b1, np.float32), (P, FEAT)).copy(),
        "b2b": np.broadcast_to(np.asarray(b2, np.float32), (P, FEAT)).copy(),
        "bfcb": np.broadcast_to(np.asarray(bfc, np.float32), (P, 2)).copy(),
        "iotaF": np.broadcast_to(np.arange(P, dtype=np.float32), (P, P)).astype(ml_dtypes.bfloat16).copy(),
        "identB": np.eye(P, dtype=ml_dtypes.bfloat16),
        "identF": np.eye(P, dtype=np.float32),
    }
    return consts


# --------------------------------------------------------------------------
# device program
# --------------------------------------------------------------------------

def build_program(cfg):
    nc = bacc.Bacc("TRN2", target_bir_lowering=False, debug=False,
                   num_devices=NCORES)

    NT = cfg.NTOT // P        # table tiles
    NB = cfg.NBLK
    TB = cfg.TBLK
    IDXW = NB * TB * 8        # idx free width

    inp = {}
    for name, shape, dt in [
        ("xT", [P, cfg.NTOT], BF16), ("ownxT", [P, cfg.NSLOT], BF16),
        ("W1p", [P, FA], BF16), ("W2p", [P, FA], BF16), ("Wfc", [P, 2], F32),
        ("b1b", [P, FEAT], F32), ("b2b", [P, FEAT], F32), ("bfcb", [P, 2], F32),
        ("iotaF", [P, P], BF16), ("identB", [P, P], BF16), ("identF", [P, P], F32),
        ("idx", [P, IDXW], I16), ("dstcol", [P, NB * TB], F32),
    ]:
        inp[name] = nc.dram_tensor(name, shape, dt, kind="ExternalInput")
    out_d = nc.dram_tensor("out", [cfg.NSLOT, 2], F32, kind="ExternalOutput")

    tab1 = nc.dram_tensor("tab1", [cfg.NTOT, ROWW], U16)
    h2own = nc.dram_tensor("h2own", [cfg.NSLOT, ROWW], U16)
    tab2 = nc.dram_tensor("tab2", [cfg.NTOT, ROWW], U16, addr_space="Shared")

    with tile.TileContext(nc) as tc:
        with (
            tc.tile_pool(name="cst", bufs=1) as cst,
            tc.tile_pool(name="sb", bufs=2) as sb,
            tc.tile_pool(name="sb3", bufs=3) as sb3,
            tc.tile_pool(name="ps", bufs=1, space="PSUM") as ps,
        ):
            nc.gpsimd.load_library(library_config.mlp)

            # ---- persistent SBUF state
            c_ = {}
            for name, shape, dt in [
                ("W1p", [P, FA], BF16), ("W2p", [P, FA], BF16), ("Wfc", [P, 2], F32),
                ("b1b", [P, FEAT], F32), ("b2b", [P, FEAT], F32), ("bfcb", [P, 2], F32),
                ("iotaF", [P, P], BF16), ("identB", [P, P], BF16),
                ("identF", [P, P], F32), ("idx", [P, IDXW], I16),
                ("dstcol", [P, NB * TB], F32),
            ]:
                t = cst.tile(shape, dt, tag=f"c_{name}")
                nc.sync.dma_start(t[:], inp[name].ap())
                c_[name] = t
            ownA1 = cst.tile([P, NB * 8], F32, tag="ownA1")
            ownA2 = cst.tile([P, NB * 8], F32, tag="ownA2")
            ownH1 = cst.tile([P, NB * P], BF16, tag="ownH1")
            ownH2 = cst.tile([P, NB * P], BF16, tag="ownH2")
            outacc = cst.tile([P, NB * 2], F32, tag="outacc")

            # ---- phase T1: full layer-1 table
            for t in range(NT):
                xt = sb3.tile([P, P], BF16, tag="xt")
                nc.sync.dma_start(xt[:], inp["xT"].ap()[:, t * P:(t + 1) * P])
                pst = ps.tile([P, FA], F32, tag="agg0")
                nc.tensor.matmul(out=pst[:], lhsT=xt[:], rhs=c_["W1p"][:],
                                 start=True, stop=True)
                row = sb3.tile([P, ROWW], U16, tag="row")
                nc.scalar.copy(row[:].bitcast(BF16)[:, 0:FEAT], pst[:, 0:FEAT])
                nc.vector.tensor_copy(row[:].bitcast(F32)[:, 64:72],
                                      pst[:, FEAT:FA])
                nc.sync.dma_start(tab1.ap()[t * P:(t + 1) * P, 0:144],
                                  row[:, 0:144])

            # ---- phase MINI1: own rows (dense self-loop path) for layer 1
            for b in range(NB):
                oxt = sb3.tile([P, P], BF16, tag="xt")
                nc.sync.dma_start(oxt[:], inp["ownxT"].ap()[:, b * P:(b + 1) * P])
                ps8 = ps.tile([P, FA], F32, tag="agg1")
                nc.tensor.matmul(out=ps8[:], lhsT=oxt[:],
                                 rhs=c_["W1p"][:], start=True, stop=True)
                nc.scalar.copy(ownH1[:, b * P:(b + 1) * P], ps8[:, 0:FEAT])
                nc.vector.tensor_copy(ownA1[:, b * 8:(b + 1) * 8],
                                      ps8[:, FEAT:FA])

            # ---- edge phases
            def edge_layer(layer, tab, ownA, ownH):
                bias = c_["b1b"] if layer == 1 else c_["b2b"]
                for b in range(NB):
                    caps = cfg.caps(b)
                    ioff = b * TB * 8
                    slabs = []
                    for c in range(4):
                        cap = caps[c]
                        slab = sb.tile([P, cap, ROWW], U16, tag=f"slab{c}")
                        co = sum(caps[:c])
                        nc.gpsimd.dma_gather(
                            out_ap=slab[:],
                            in_ap=tab.ap()[c * cfg.CHUNK:(c + 1) * cfg.CHUNK, :],
                            idxs_ap=c_["idx"][:, ioff + co * 8: ioff + (co + cap) * 8],
                            num_idxs=cap * P, num_idxs_reg=cap * P,
                            elem_size=ROWW,
                        )
                        slabs.append(slab)

                    # loop1: indicators + alpha_dst expand
                    m2blk = sb.tile([P, TB * P], BF16, tag="m2blk")
                    psad = ps.tile([P, TB * 4], F32, tag=f"ad{b % 2}")
                    for t in range(TB):
                        nc.vector.tensor_scalar(
                            out=m2blk[:, t * P:(t + 1) * P], in0=c_["iotaF"][:],
                            scalar1=c_["dstcol"][:, b * TB + t: b * TB + t + 1],
                            scalar2=None, op0=OP.is_equal)
                        psm1 = ps.tile([P, P], BF16, tag=f"m1ps{t % 2}")
                        nc.tensor.transpose(out=psm1[:],
                                            in_=m2blk[:, t * P:(t + 1) * P],
                                            identity=c_["identB"][:])
                        m1 = sb.tile([P, P], F32, tag=f"m1_{t % 2}")
                        nc.scalar.copy(m1[:], psm1[:])
                        nc.tensor.matmul(
                            out=psad[:, t * 4:(t + 1) * 4], lhsT=m1[:],
                            rhs=ownA[:, b * 8 + 4:(b + 1) * 8],
                            start=True, stop=True)

                    # batched logits -> exp weights
                    e1 = sb.tile([P, TB, 4], F32, tag="e1")
                    for c in range(4):
                        co = sum(caps[:c])
                        cap = caps[c]
                        nc.vector.tensor_tensor(
                            out=e1[:, co:co + cap, :],
                            in0=slabs[c][:].bitcast(F32)[:, :, 64:68],
                            in1=psad[:, co * 4:(co + cap) * 4]
                                .rearrange("p (a b) -> p a b", b=4),
                            op=OP.add)
                    eA = sb.tile([P, TB, 4], F32, tag="eA")
                    nc.scalar.activation(eA[:], e1[:], AF.Exp, scale=0.2)
                    rl = sb.tile([P, TB, 4], F32, tag="rl")
                    nc.scalar.activation(rl[:], e1[:], AF.Relu)
                    eB = sb.tile([P, TB, 4], F32, tag="eB")
                    nc.scalar.activation(eB[:], rl[:], AF.Exp, scale=0.8)
                    expq = sb.tile([P, TB, 4], BF16, tag="expq")
                    nc.vector.tensor_tensor(out=expq[:], in0=eA[:], in1=eB[:],
                                            op=OP.mult)

                    # weighted rows (exp-scaled h | exp) per slab
                    wsl = []
                    for c in range(4):
                        co = sum(caps[:c])
                        cap = caps[c]
                        w = sb.tile([P, cap, FEAT + 4], BF16, tag=f"w{c}")
                        nc.vector.tensor_tensor(
                            out=w[:, :, 0:FEAT].rearrange("p a (b c) -> p a b c", b=4),
                            in0=slabs[c][:].bitcast(BF16)[:, :, 0:FEAT]
                                .rearrange("p a (b c) -> p a b c", b=4),
                            in1=expq[:, co:co + cap, :]
                                .to_broadcast([P, cap, 4, HID]),
                            op=OP.mult)
                        nc.vector.tensor_copy(w[:, :, FEAT:FEAT + 4],
                                              expq[:, co:co + cap, :])
                        wsl.append(w)

                    # aggregation (numerator | denominator in one rhs)
                    psagg = ps.tile([P, FEAT + 4], F32, tag=f"agg{b % 2}")
                    t = 0
                    for c in range(4):
                        for j in range(caps[c]):
                            nc.tensor.matmul(
                                out=psagg[:], lhsT=m2blk[:, t * P:(t + 1) * P],
                                rhs=wsl[c][:, j, :],
                                start=(t == 0), stop=(t == TB - 1))
                            t += 1

                    # dense self-loop contribution
                    es = sb.tile([P, 4], F32, tag="es")
                    nc.vector.tensor_tensor(out=es[:], in0=ownA[:, b * 8:b * 8 + 4],
                                            in1=ownA[:, b * 8 + 4:b * 8 + 8],
                                            op=OP.add)
                    sA = sb.tile([P, 4], F32, tag="sA")
                    nc.scalar.activation(sA[:], es[:], AF.Exp, scale=0.2)
                    sR = sb.tile([P, 4], F32, tag="sR")
                    nc.scalar.activation(sR[:], es[:], AF.Relu)
                    sB = sb.tile([P, 4], F32, tag="sB")
                    nc.scalar.activation(sB[:], sR[:], AF.Exp, scale=0.8)
                    expS = sb.tile([P, 4], F32, tag="expS")
                    nc.vector.tensor_tensor(out=expS[:], in0=sA[:], in1=sB[:],
                                            op=OP.mult)
                    hof = sb.tile([P, FEAT], F32, tag="hof")
                    nc.vector.tensor_copy(hof[:], ownH[:, b * P:(b + 1) * P])
                    numer = sb.tile([P, FEAT], F32, tag="numer")
                    nc.vector.tensor_tensor(
                        out=numer[:].rearrange("p (a b) -> p a b", b=HID),
                        in0=hof[:].rearrange("p (a b) -> p a b", b=HID),
                        in1=expS[:].to_broadcast([P, 4, HID]), op=OP.mult)
                    nc.vector.tensor_tensor(out=numer[:], in0=psagg[:, 0:FEAT],
                                            in1=numer[:], op=OP.add)

                    # normalize + bias + elu
                    den = sb.tile([P, 4], F32, tag="den")
                    nc.vector.tensor_tensor(out=den[:], in0=psagg[:, FEAT:FEAT + 4],
                                            in1=expS[:], op=OP.add)
                    rec = sb.tile([P, 4], F32, tag="rec")
                    nc.vector.reciprocal(rec[:], den[:])
                    zb = sb.tile([P, FEAT], F32, tag="zb")
                    nc.vector.tensor_tensor(
                        out=zb[:].rearrange("p (a b) -> p a b", b=HID),
                        in0=numer[:].rearrange("p (a b) -> p a b", b=HID),
                        in1=rec[:].to_broadcast([P, 4, HID]), op=OP.mult)
                    nc.vector.tensor_tensor(out=zb[:], in0=zb[:], in1=bias[:],
                                            op=OP.add)
                    rz = sb.tile([P, FEAT], F32, tag="rz")
                    nc.scalar.activation(rz[:], zb[:], AF.Relu)
                    zm = sb.tile([P, FEAT], F32, tag="zm")
                    nc.vector.tensor_tensor(out=zm[:], in0=zb[:], in1=rz[:],
                                            op=OP.subtract)
                    em = sb.tile([P, FEAT], F32, tag="em")
                    nc.scalar.activation(em[:], zm[:], AF.Exp)
                    yt = sb.tile([P, FEAT], F32, tag="yt")
                    nc.vector.tensor_tensor(out=yt[:], in0=em[:], in1=rz[:],
                                            op=OP.add)

                    if layer == 1:
                        y1b = sb.tile([P, FEAT], BF16, tag="y1b")
                        nc.vector.tensor_scalar(out=y1b[:], in0=yt[:],
                                                scalar1=-1.0, scalar2=None,
                                                op0=OP.add)
                        psyt = ps.tile([P, P], BF16, tag="epi_a")
                        nc.tensor.transpose(out=psyt[:], in_=y1b[:],
                                            identity=c_["identB"][:])
                        y1T = sb.tile([P, P], BF16, tag="y1T")
                        nc.scalar.copy(y1T[:], psyt[:])
                        psh2 = ps.tile([P, FA], F32, tag="epi_b")
                        nc.tensor.matmul(out=psh2[:], lhsT=y1T[:],
                                         rhs=c_["W2p"][:], start=True, stop=True)
                        row2 = sb.tile([P, ROWW], U16, tag="row2")
                        nc.scalar.copy(row2[:].bitcast(BF16)[:, 0:FEAT],
                                       psh2[:, 0:FEAT])
                        nc.vector.tensor_copy(row2[:].bitcast(F32)[:, 64:72],
                                              psh2[:, FEAT:FA])
                        nc.vector.tensor_copy(ownA2[:, b * 8:(b + 1) * 8],
                                              psh2[:, FEAT:FA])
                        nc.scalar.copy(ownH2[:, b * P:(b + 1) * P],
                                       psh2[:, 0:FEAT])
                        nc.sync.dma_start(h2own.ap()[b * P:(b + 1) * P, 0:144],
                                          row2[:, 0:144])
                    else:
                        y2f = sb.tile([P, FEAT], F32, tag="y2f")
                        nc.vector.tensor_scalar(out=y2f[:], in0=yt[:],
                                                scalar1=-1.0, scalar2=None,
                                                op0=OP.add)
                        psyt2 = ps.tile([P, P], F32, tag="epi_a")
                        nc.tensor.transpose(out=psyt2[:], in_=y2f[:],
                                            identity=c_["identF"][:])
                        y2T = sb.tile([P, P], F32, tag="y1T")
                        nc.scalar.copy(y2T[:], psyt2[:])
                        psfc = ps.tile([P, 2], F32, tag="epi_b")
                        nc.tensor.matmul(out=psfc[:], lhsT=y2T[:],
                                         rhs=c_["Wfc"][:], start=True, stop=True)
                        nc.vector.tensor_tensor(out=outacc[:, b * 2:(b + 1) * 2],
                                                in0=psfc[:], in1=c_["bfcb"][:],
                                                op=OP.add)

            edge_layer(1, tab1, ownA1, ownH1)

            nc.gpsimd.collective_compute(
                "AllGather", OP.bypass,
                replica_groups=[list(range(NCORES))],
                ins=[h2own.ap().opt()], outs=[tab2.ap().opt()])

            edge_layer(2, tab2, ownA2, ownH2)

            nc.sync.dma_start(
                out_d.ap().rearrange("(b p) o -> p b o", p=P),
                outacc[:].rearrange("p (b o) -> p b o", o=2))

    nc.compile()
    return nc


# --------------------------------------------------------------------------
# top-level entry
# --------------------------------------------------------------------------

_CACHE = {}


def _get_program(cfg):
    key = (cfg.N, cfg.NBLK, tuple(cfg.caps_base))
    if key not in _CACHE:
        t0 = time.time()
        _CACHE[key] = build_program(cfg)
        print(f"[build+compile] {time.time()-t0:.1f}s", flush=True)
    return _CACHE[key]


def run(cfg, inputs, trace=False):
    x = np.asarray(inputs["x"], np.float32)
    packed = pack(cfg, x, inputs["edge_index"])
    consts = make_weights(cfg, inputs["W1"], inputs["a_src1"], inputs["a_dst1"],
                          inputs["b1"], inputs["W2"], inputs["a_src2"],
                          inputs["a_dst2"], inputs["b2"], inputs["Wfc"],
                          inputs["bfc"])
    nc = _get_program(cfg)

    in_maps = []
    for k in range(NCORES):
        m = dict(consts)
        m["xT"] = packed["xT"]
        m["ownxT"] = np.ascontiguousarray(
            packed["xT"][:, k * cfg.NSLOT:(k + 1) * cfg.NSLOT])
        m["idx"] = packed["idx"][k]
        m["dstcol"] = packed["dstcol"][k]
        in_maps.append(m)

    res = run_bass_kernel_spmd(nc, in_maps, core_ids=list(range(NCORES)),
                               trace=trace)
    outs = np.concatenate([r["out"] for r in res.results], axis=0)  # [NTOT, 2]
    full = np.zeros((cfg.N, 2), np.float32)
    mask = packed["node_of_slot"] >= 0
    full[packed["node_of_slot"][mask]] = outs[mask]
    return full, res


def kernel(**inputs):
    out, _ = run(REAL, inputs)
    return out

